# revision 6
# baseline (speedup 1.0000x reference)
"""GCNConv (X @ W, then unweighted CSR neighbor-sum) on 8 TRN2 NeuronCores.

Strategy (hardcoded for N=50000, E=800000, D_in=128, D_out=64, 8 cores):
  - Destination nodes are sharded: core k owns rows [6250k, 6250(k+1)).
    Edges follow their (sorted) destination row, so each core gets a
    contiguous slice of the edge list.  The weight matrix is replicated.
  - Host preprocessing is index manipulation + layout only: the edge
    shard's required neighbor features are materialized per lane
    (Xg[lane] = X[col[e]], bf16, lane-major) -- the halo for this
    core's edge partition.  All FLOPs on tensor data happen on device.
  - Device: stream Xg in ~2MB contiguous chunks (HWDGE, full HBM BW).
    Aggregation runs in D_in space BEFORE the dense transform
    (out = (A^T Xg) @ W): per 64-dest block b, the segment sum is a
    collision-free one-hot matmul S_b^T[128f,64d] += Xg_t^T @ M_t with
    M_t[lane,dest] = (rowrel == iota), accumulated in PSUM over the
    block's edge tiles.  Then one [64x64] matmul out_b = S_b @ W.
    No GPSIMD/indirect DMA anywhere (the v1 kernel spent 75% of its
    time on per-tile SWDGE fixed overhead).
"""

import numpy as np
import ml_dtypes

import concourse.bass as bass
import concourse.mybir as mybir
import concourse.tile as tile
from concourse import bacc
from concourse.bass_utils import run_bass_kernel_spmd

# ---- problem constants (must match the harness inputs) ----
N_NODES = 50000
N_EDGES = 800000
D_IN = 128
D_OUT = 64
N_CORES = 8

NODES_PER_CORE = N_NODES // N_CORES            # 6250
BLK = 64                                       # dest-block width (matmul N dim)
BLOCKS_PER_CORE = (NODES_PER_CORE + BLK - 1) // BLK   # 98
CB = 7                                         # dest blocks per streamed chunk
N_CHUNKS = BLOCKS_PER_CORE // CB               # 14

ST_DT = mybir.dt.bfloat16
NP_ST = ml_dtypes.bfloat16

# test.py can flip this to get a profiled run; results land in LAST_RESULTS.
TRACE = False
LAST_RESULTS = None


def build_program(T_list):
    """One SPMD program shared by all 8 cores (per-core variation is data).

    T_list[b] = edge tiles for dest block b (uniform across cores).
    """
    T_list = [int(t) for t in T_list]
    NT = int(sum(T_list))                      # edge tiles per core
    off = np.concatenate([[0], np.cumsum(T_list)]).astype(int)

    nc = bacc.Bacc("TRN2", target_bir_lowering=False, debug=False,
                   num_devices=N_CORES)
    xg = nc.dram_tensor("xg", [128, NT * 128], ST_DT,
                        kind="ExternalInput").ap()
    rr = nc.dram_tensor("rr", [128, NT], ST_DT, kind="ExternalInput").ap()
    w = nc.dram_tensor("w", [D_IN, D_OUT], ST_DT, kind="ExternalInput").ap()
    iota = nc.dram_tensor("iota", [128, BLK], ST_DT,
                          kind="ExternalInput").ap()
    # output laid [dest_in_block, block, feat]; host transposes to [node, feat]
    out = nc.dram_tensor("out", [BLK, BLOCKS_PER_CORE, D_OUT],
                         mybir.dt.float32, kind="ExternalOutput").ap()

    with tile.TileContext(nc) as tc:
        with (
            tc.tile_pool(name="const", bufs=1) as cpool,
            tc.tile_pool(name="xg", bufs=4) as xgpool,
            tc.tile_pool(name="msk", bufs=3) as mpool,
            tc.tile_pool(name="agg", bufs=6, space="PSUM") as apsum,
            tc.tile_pool(name="sal", bufs=1) as spool,
            tc.tile_pool(name="ops", bufs=2, space="PSUM") as opsum,
            tc.tile_pool(name="ob", bufs=3) as opool,
        ):
            # ---- constants ----
            w_sb = cpool.tile([D_IN, D_OUT], ST_DT)
            nc.sync.dma_start(w_sb[:], w[:])
            iota_sb = cpool.tile([128, BLK], ST_DT)
            nc.sync.dma_start(iota_sb[:], iota[:])
            rr_sb = cpool.tile([128, NT], ST_DT)
            nc.sync.dma_start(rr_sb[:], rr[:])

            # all 98 aggregated S_b^T columns live in SBUF until transformed
            s_all = spool.tile([D_IN, BLOCKS_PER_CORE, BLK], ST_DT)

            def emit_transform(cj):
                """transform + store chunk cj's blocks (inputs long ready)."""
                b0 = cj * CB
                pp = opsum.tile([BLK, CB * D_OUT], mybir.dt.float32)
                for b in range(CB):
                    nc.tensor.matmul(
                        out=pp[:, b * D_OUT:(b + 1) * D_OUT],
                        lhsT=s_all[:, b0 + b, :], rhs=w_sb[:],
                        start=True, stop=True)
                ob_t = opool.tile([BLK, CB, D_OUT], mybir.dt.float32)
                nc.scalar.copy(
                    ob_t[:], pp[:].rearrange("d (b f) -> d b f", f=D_OUT))
                nc.sync.dma_start(out[:, b0:b0 + CB, :], ob_t[:])

            def emit_mask(cj):
                """one-hot masks for chunk cj's tiles (DVE, ~4us) -- issued a
                chunk ahead so they overlap the previous chunk's matmuls."""
                t0 = int(off[cj * CB])
                ntc = int(off[(cj + 1) * CB]) - t0
                m_t = mpool.tile([128, ntc, BLK], ST_DT, tag="m")
                nc.vector.tensor_tensor(
                    out=m_t[:],
                    in0=rr_sb[:, t0:t0 + ntc].unsqueeze(2).to_broadcast(
                        [128, ntc, BLK]),
                    in1=iota_sb[:].unsqueeze(1).to_broadcast(
                        [128, ntc, BLK]),
                    op=mybir.AluOpType.is_equal)
                return m_t

            m_cur = emit_mask(0)
            for ci in range(N_CHUNKS):
                b0 = ci * CB                   # first block of chunk
                t0 = int(off[b0])              # first edge tile of chunk
                ntc = int(off[b0 + CB]) - t0
                xg_t = xgpool.tile([128, ntc * 128], ST_DT)
                nc.sync.dma_start(xg_t[:], xg[:, t0 * 128:(t0 + ntc) * 128])
                m_t, m_cur = m_cur, (emit_mask(ci + 1)
                                     if ci + 1 < N_CHUNKS else None)
                for b in range(CB):
                    gb = b0 + b                # global block id on this core
                    Tb = int(T_list[gb])
                    ps = apsum.tile([D_IN, BLK], mybir.dt.float32)
                    for t in range(Tb):
                        ti = int(off[gb]) - t0 + t
                        nc.tensor.matmul(
                            out=ps[:],
                            lhsT=xg_t[:, ti * 128:(ti + 1) * 128],
                            rhs=m_t[:, ti, :],
                            start=(t == 0), stop=(t == Tb - 1))
                    nc.scalar.copy(s_all[:, gb, :], ps[:])
                if ci > 0:
                    emit_transform(ci - 1)
            emit_transform(N_CHUNKS - 1)

    nc.compile()
    return nc


def prepare_inputs(X, weights, row_index, column_index):
    """Host-side shard/pad/layout: per-core per-block edge tiling, halo
    materialization (gather of X rows per edge lane), and transposes."""
    row = np.ascontiguousarray(row_index).astype(np.int64)
    col = np.ascontiguousarray(column_index).astype(np.int64)
    core_bounds = np.searchsorted(
        row, np.arange(N_CORES + 1) * NODES_PER_CORE)

    X_bf = np.ascontiguousarray(X).astype(NP_ST)
    w_np = np.ascontiguousarray(weights).astype(NP_ST)
    iota_np = np.broadcast_to(
        np.arange(BLK, dtype=np.float32), (128, BLK)).astype(NP_ST)

    # per-core, per-block edge counts -> uniform tile counts
    cores = []
    EB = np.zeros((N_CORES, BLOCKS_PER_CORE), dtype=np.int64)
    for k in range(N_CORES):
        lo, hi = core_bounds[k], core_bounds[k + 1]
        r = row[lo:hi] - k * NODES_PER_CORE
        c = col[lo:hi]
        bb = np.searchsorted(r, np.arange(BLOCKS_PER_CORE + 1) * BLK)
        EB[k] = bb[1:] - bb[:-1]
        cores.append((r, c, bb))
    T_list = np.maximum((EB.max(axis=0) + 127) // 128, 1)
    off = np.concatenate([[0], np.cumsum(T_list)]).astype(np.int64)
    NT = int(off[-1])
    NL = NT * 128

    in_maps = []
    for k in range(N_CORES):
        r, c, bb = cores[k]
        cols_flat = np.zeros(NL, dtype=np.int64)
        rr = np.full(NL, -1.0, dtype=np.float32)
        valid = np.zeros(NL, dtype=bool)
        for b in range(BLOCKS_PER_CORE):
            s, e = bb[b], bb[b + 1]
            base = int(off[b]) * 128
            cols_flat[base:base + (e - s)] = c[s:e]
            rr[base:base + (e - s)] = (r[s:e] - b * BLK).astype(np.float32)
            valid[base:base + (e - s)] = True
        # lane-major halo: xg[l, t*128+f] = X[col[e(t,l)], f]
        A = X_bf[cols_flat]                       # [NT*128, 128]
        A[~valid] = 0
        xg_k = np.ascontiguousarray(
            A.reshape(NT, 128, D_IN).transpose(1, 0, 2).reshape(128, NT * 128))
        in_maps.append({
            "xg": xg_k,
            "rr": np.ascontiguousarray(
                rr.reshape(NT, 128).T).astype(NP_ST),
            "w": w_np,
            "iota": iota_np,
        })
    return T_list, in_maps


def kernel(X, weights, row_index, column_index):
    global LAST_RESULTS
    T_list, in_maps = prepare_inputs(X, weights, row_index, column_index)
    nc = build_program(T_list)
    res = run_bass_kernel_spmd(nc, in_maps, list(range(N_CORES)),
                               trace=TRACE)
    LAST_RESULTS = res
    # device out is [dest_in_block, block, feat] -> [node, feat]
    out = np.concatenate(
        [res.results[k]["out"].transpose(1, 0, 2).reshape(-1, D_OUT)
         [:NODES_PER_CORE] for k in range(N_CORES)],
        axis=0)
    return out.astype(np.float32)


# revision 9
# speedup vs baseline: 1.1594x; 1.1594x over previous
"""GCNConv (X @ W, then unweighted CSR neighbor-sum) on 8 TRN2 NeuronCores.

Strategy (hardcoded for N=50000, E=800000, D_in=128, D_out=64, 8 cores):
  - Destination nodes are sharded: core k owns rows [6250k, 6250(k+1)).
    Edges follow their (sorted) destination row, so each core gets a
    contiguous slice of the edge list.  The weight matrix is replicated.
  - Host preprocessing is index manipulation + layout only: the edge
    shard's required neighbor features are materialized per lane
    (Xg[lane] = X[col[e]], bf16, lane-major) -- the halo for this
    core's edge partition.  All FLOPs on tensor data happen on device.
  - Device: stream Xg in ~2MB contiguous chunks (HWDGE, full HBM BW).
    Aggregation runs in D_in space BEFORE the dense transform
    (out = (A^T Xg) @ W): per 64-dest block b, the segment sum is a
    collision-free one-hot matmul S_b^T[128f,64d] += Xg_t^T @ M_t with
    M_t[lane,dest] = (rowrel == iota), accumulated in PSUM over the
    block's edge tiles.  Then one [64x64] matmul out_b = S_b @ W.
    No GPSIMD/indirect DMA anywhere (the v1 kernel spent 75% of its
    time on per-tile SWDGE fixed overhead).
"""

import numpy as np
import ml_dtypes

import concourse.bass as bass
import concourse.mybir as mybir
import concourse.tile as tile
from concourse import bacc
from concourse.bass_utils import run_bass_kernel_spmd

# ---- problem constants (must match the harness inputs) ----
N_NODES = 50000
N_EDGES = 800000
D_IN = 128
D_OUT = 64
N_CORES = 8

NODES_PER_CORE = N_NODES // N_CORES            # 6250
BLK = 64                                       # dest-block width (matmul N dim)
BLOCKS_PER_CORE = (NODES_PER_CORE + BLK - 1) // BLK   # 98
CB = 7                                         # dest blocks per streamed chunk
N_CHUNKS = BLOCKS_PER_CORE // CB               # 14

ST_DT = mybir.dt.bfloat16
NP_ST = ml_dtypes.bfloat16

# test.py can flip this to get a profiled run; results land in LAST_RESULTS.
TRACE = False
LAST_RESULTS = None


def build_program(T_list):
    """One SPMD program shared by all 8 cores (per-core variation is data).

    T_list[b] = edge tiles for dest block b (uniform across cores).
    """
    T_list = [int(t) for t in T_list]
    NT = int(sum(T_list))                      # edge tiles per core
    off = np.concatenate([[0], np.cumsum(T_list)]).astype(int)

    nc = bacc.Bacc("TRN2", target_bir_lowering=False, debug=False,
                   num_devices=N_CORES)
    xg = nc.dram_tensor("xg", [128, NT * 128], ST_DT,
                        kind="ExternalInput").ap()
    rr = nc.dram_tensor("rr", [128, NT], ST_DT, kind="ExternalInput").ap()
    w = nc.dram_tensor("w", [D_IN, D_OUT], ST_DT, kind="ExternalInput").ap()
    iota = nc.dram_tensor("iota", [128, BLK], ST_DT,
                          kind="ExternalInput").ap()
    # output laid [dest_in_block, block, feat]; host transposes to [node, feat]
    out = nc.dram_tensor("out", [BLK, BLOCKS_PER_CORE, D_OUT],
                         mybir.dt.float32, kind="ExternalOutput").ap()

    with tile.TileContext(nc) as tc:
        with (
            tc.tile_pool(name="const", bufs=1) as cpool,
            tc.tile_pool(name="xg", bufs=4) as xgpool,
            tc.tile_pool(name="msk", bufs=3) as mpool,
            tc.tile_pool(name="agg", bufs=6, space="PSUM") as apsum,
            tc.tile_pool(name="sal", bufs=1) as spool,
            tc.tile_pool(name="ops", bufs=2, space="PSUM") as opsum,
            tc.tile_pool(name="ob", bufs=3) as opool,
        ):
            # ---- constants ----
            w_sb = cpool.tile([D_IN, D_OUT], ST_DT)
            nc.sync.dma_start(w_sb[:], w[:])
            iota_sb = cpool.tile([128, BLK], ST_DT)
            nc.sync.dma_start(iota_sb[:], iota[:])
            rr_sb = cpool.tile([128, NT], ST_DT)
            nc.sync.dma_start(rr_sb[:], rr[:])

            # all 98 aggregated S_b^T columns live in SBUF until transformed
            s_all = spool.tile([D_IN, BLOCKS_PER_CORE, BLK], ST_DT)

            def emit_transform(cj):
                """transform + store chunk cj's blocks (inputs long ready)."""
                b0 = cj * CB
                pp = opsum.tile([BLK, CB * D_OUT], mybir.dt.float32)
                for b in range(CB):
                    nc.tensor.matmul(
                        out=pp[:, b * D_OUT:(b + 1) * D_OUT],
                        lhsT=s_all[:, b0 + b, :], rhs=w_sb[:],
                        start=True, stop=True)
                ob_t = opool.tile([BLK, CB, D_OUT], mybir.dt.float32)
                nc.scalar.copy(
                    ob_t[:], pp[:].rearrange("d (b f) -> d b f", f=D_OUT))
                # out DMA on the ACT queue: keeps the SP queue free for xg
                # prefetches (SP must never wait on the copy/transform chain)
                nc.scalar.dma_start(out[:, b0:b0 + CB, :], ob_t[:])

            def emit_mask(cj):
                """one-hot masks for chunk cj's tiles (DVE, ~4us) -- issued a
                chunk ahead so they overlap the previous chunk's matmuls."""
                t0 = int(off[cj * CB])
                ntc = int(off[(cj + 1) * CB]) - t0
                m_t = mpool.tile([128, ntc, BLK], ST_DT, tag="m")
                nc.vector.tensor_tensor(
                    out=m_t[:],
                    in0=rr_sb[:, t0:t0 + ntc].unsqueeze(2).to_broadcast(
                        [128, ntc, BLK]),
                    in1=iota_sb[:].unsqueeze(1).to_broadcast(
                        [128, ntc, BLK]),
                    op=mybir.AluOpType.is_equal)
                return m_t

            m_cur = emit_mask(0)
            for ci in range(N_CHUNKS):
                b0 = ci * CB                   # first block of chunk
                t0 = int(off[b0])              # first edge tile of chunk
                ntc = int(off[b0 + CB]) - t0
                xg_t = xgpool.tile([128, ntc * 128], ST_DT)
                nc.sync.dma_start(xg_t[:], xg[:, t0 * 128:(t0 + ntc) * 128])
                m_t, m_cur = m_cur, (emit_mask(ci + 1)
                                     if ci + 1 < N_CHUNKS else None)
                if ci > 0:
                    # transform chunk ci-1 now: every dep is a chunk old, so
                    # these run stall-free ahead of chunk ci's agg matmuls
                    emit_transform(ci - 1)
                for b in range(CB):
                    gb = b0 + b                # global block id on this core
                    Tb = int(T_list[gb])
                    ps = apsum.tile([D_IN, BLK], mybir.dt.float32)
                    for t in range(Tb):
                        ti = int(off[gb]) - t0 + t
                        nc.tensor.matmul(
                            out=ps[:],
                            lhsT=xg_t[:, ti * 128:(ti + 1) * 128],
                            rhs=m_t[:, ti, :],
                            start=(t == 0), stop=(t == Tb - 1))
                    nc.scalar.copy(s_all[:, gb, :], ps[:])
            emit_transform(N_CHUNKS - 1)

    nc.compile()
    return nc


def prepare_inputs(X, weights, row_index, column_index):
    """Host-side shard/pad/layout: per-core per-block edge tiling, halo
    materialization (gather of X rows per edge lane), and transposes."""
    row = np.ascontiguousarray(row_index).astype(np.int64)
    col = np.ascontiguousarray(column_index).astype(np.int64)
    core_bounds = np.searchsorted(
        row, np.arange(N_CORES + 1) * NODES_PER_CORE)

    X_bf = np.ascontiguousarray(X).astype(NP_ST)
    w_np = np.ascontiguousarray(weights).astype(NP_ST)
    iota_np = np.broadcast_to(
        np.arange(BLK, dtype=np.float32), (128, BLK)).astype(NP_ST)

    # per-core, per-block edge counts -> uniform tile counts
    cores = []
    EB = np.zeros((N_CORES, BLOCKS_PER_CORE), dtype=np.int64)
    for k in range(N_CORES):
        lo, hi = core_bounds[k], core_bounds[k + 1]
        r = row[lo:hi] - k * NODES_PER_CORE
        c = col[lo:hi]
        bb = np.searchsorted(r, np.arange(BLOCKS_PER_CORE + 1) * BLK)
        EB[k] = bb[1:] - bb[:-1]
        cores.append((r, c, bb))
    T_list = np.maximum((EB.max(axis=0) + 127) // 128, 1)
    off = np.concatenate([[0], np.cumsum(T_list)]).astype(np.int64)
    NT = int(off[-1])
    NL = NT * 128

    in_maps = []
    for k in range(N_CORES):
        r, c, bb = cores[k]
        cols_flat = np.zeros(NL, dtype=np.int64)
        rr = np.full(NL, -1.0, dtype=np.float32)
        valid = np.zeros(NL, dtype=bool)
        for b in range(BLOCKS_PER_CORE):
            s, e = bb[b], bb[b + 1]
            base = int(off[b]) * 128
            cols_flat[base:base + (e - s)] = c[s:e]
            rr[base:base + (e - s)] = (r[s:e] - b * BLK).astype(np.float32)
            valid[base:base + (e - s)] = True
        # lane-major halo: xg[l, t*128+f] = X[col[e(t,l)], f]
        A = X_bf[cols_flat]                       # [NT*128, 128]
        A[~valid] = 0
        xg_k = np.ascontiguousarray(
            A.reshape(NT, 128, D_IN).transpose(1, 0, 2).reshape(128, NT * 128))
        in_maps.append({
            "xg": xg_k,
            "rr": np.ascontiguousarray(
                rr.reshape(NT, 128).T).astype(NP_ST),
            "w": w_np,
            "iota": iota_np,
        })
    return T_list, in_maps


def kernel(X, weights, row_index, column_index):
    global LAST_RESULTS
    T_list, in_maps = prepare_inputs(X, weights, row_index, column_index)
    nc = build_program(T_list)
    res = run_bass_kernel_spmd(nc, in_maps, list(range(N_CORES)),
                               trace=TRACE)
    LAST_RESULTS = res
    # device out is [dest_in_block, block, feat] -> [node, feat]
    out = np.concatenate(
        [res.results[k]["out"].transpose(1, 0, 2).reshape(-1, D_OUT)
         [:NODES_PER_CORE] for k in range(N_CORES)],
        axis=0)
    return out.astype(np.float32)
